# revision 14
# baseline (speedup 1.0000x reference)
"""Sparse (adjacency-masked) multi-head attention for Trainium2, 8 cores.

Problem: b=4, s=2048, e=512, h=8 heads, d=64.
  qkv = x @ Wqkv^T + b -> q,k,v per head
  scores = (q @ k^T) / sqrt(d) * adj   (multiplicative 0/1 mask, clip is a no-op)
  attn = softmax(scores); out = (attn @ v) reshaped @ out_w^T + out_b

Sharding: core c -> batch c//2, local heads [4*(c%2), 4*(c%2)+4).  Each core
computes a partial out-projection over its 4 heads; host sums the two
partials per batch and adds the (host-folded) biases.  No collectives.

Key device-side formulation (all matmuls bf16 -> fp32 PSUM):
  - Everything is computed transposed: S^T[k,q] = k^T(stationary) . q^T so the
    mask ships host-transposed; U^T = exp(S^T) with masked entries forced to
    exp(0)=1 via copy_predicated (matches reference: masked score 0 -> exp 1).
  - attn numerator+denominator in one matmul: lhsT = [v_h | 1] (M=65), so PSUM
    row 64 is the softmax denominator.
  - v bias never touches the device: softmax rows sum to 1, so +bv shifts the
    attention output by a constant vector; it is folded into the final bias on
    host as (bv_full @ out_w^T) + out_b.
"""

import numpy as np

import concourse.bass as bass
import concourse.tile as tile
from concourse import bacc, mybir
from concourse.bass_utils import run_bass_kernel_spmd

BF16 = mybir.dt.bfloat16
F32 = mybir.dt.float32

# Problem constants (hardcoded per contract)
B, S, E = 4, 2048, 512
H_TOT, D = 8, 64
HL = 4            # local heads per core
N_CORES = 8

_CACHED_NC = None


def build_kernel(s=S, e=E, hl=HL, d=D, qb_size=512):
    """Per-core SPMD kernel. Inputs (per core):
      xT     [e, s]        bf16  (x[b].T)
      wqkT   [e, 4, 128]   bf16  pair-blocks pb: 0=[q_h0;q_h1] 1=[q_h2;q_h3]
                                 2=[k_h0;k_h1] 3=[k_h2;k_h3]; q pre-scaled 1/sqrt(d)
      bqk    [128, 4]      f32   bias rows matching wqkT blocks (q pre-scaled)
      wvT    [e, hl*d]     bf16  v weights, local-head-major columns
      woT    [d, hl, e]    bf16  out_w slice transposed per local head
      notaT  [s, s]        bf16  (1 - adj[b]).T  (indexed [k, q])
    Output:
      part   [s, e]        f32   partial out-projection (no bias)
    """
    assert e % 128 == 0 and s % 128 == 0
    EC = e // 128                 # contraction chunks for projections
    n_qb = s // qb_size           # q blocks
    n_kc = s // 128               # k chunks
    n_st = s // 128               # s tiles for v / proj

    nc = bacc.Bacc(None, target_bir_lowering=False)

    xT_d = nc.dram_tensor("xT", [e, s], BF16, kind="ExternalInput")
    wqkT_d = nc.dram_tensor("wqkT", [e, 4, 128], BF16, kind="ExternalInput")
    bqk_d = nc.dram_tensor("bqk", [128, 4], F32, kind="ExternalInput")
    wvT_d = nc.dram_tensor("wvT", [e, hl * d], BF16, kind="ExternalInput")
    woT_d = nc.dram_tensor("woT", [d, hl, e], BF16, kind="ExternalInput")
    notaT_d = nc.dram_tensor("notaT", [s, s], mybir.dt.uint16, kind="ExternalInput")
    part_d = nc.dram_tensor("part", [s, e], F32, kind="ExternalOutput")

    with tile.TileContext(nc) as tc:
        with (
            tc.tile_pool(name="singles", bufs=1) as singles,
            tc.tile_pool(name="nota", bufs=3) as nota_pool,
            tc.tile_pool(name="upool", bufs=3) as u_pool,
            tc.tile_pool(name="small", bufs=4) as small,
            tc.tile_pool(name="dbounce", bufs=4, space="DRAM") as dbounce,
            tc.tile_pool(name="outbuf", bufs=3) as outbuf,
            tc.tile_pool(name="ps_big", bufs=1, space="PSUM") as ps_big,
            tc.tile_pool(name="ps_o", bufs=1, space="PSUM") as ps_o,
        ):
            # ---- resident tensors -------------------------------------
            xT_s = singles.tile([128, EC, s], BF16)
            nc.sync.dma_start(xT_s[:], xT_d.rearrange("(eo ei) s -> ei eo s", ei=128))
            wqkT_s = singles.tile([128, EC, 4, 128], BF16)
            nc.sync.dma_start(
                wqkT_s[:], wqkT_d.rearrange("(eo ei) pb j -> ei eo pb j", ei=128)
            )
            bqk_s = singles.tile([128, 4], F32)
            nc.sync.dma_start(bqk_s[:], bqk_d[:])
            wvT_s = singles.tile([128, EC, hl * d], BF16)
            nc.sync.dma_start(
                wvT_s[:], wvT_d.rearrange("(eo ei) f -> ei eo f", ei=128)
            )
            woT_s = singles.tile([d, hl, e], BF16)
            nc.sync.dma_start(woT_s[:], woT_d[:])

            ones_s = singles.tile([128, 4 * qb_size], BF16)
            nc.vector.memset(ones_s[:], 1.0)

            # qkT pair-blocks: [128, 4, s]; head h lives at partitions
            # 64*(h%2) .. +64 of block (h//2) [q] / 2+(h//2) [k]
            qkT_s = singles.tile([128, 4, s], BF16)
            # v augmented with a ones column: [128, st, h, d+1]
            vaug_s = singles.tile([128, n_st, hl, d + 1], BF16)
            nc.vector.memset(vaug_s[:], 1.0)
            # normalized attn output, transposed: [d, h, s] (partitions 0:d)
            outT_s = singles.tile([d, hl, s], BF16)

            # ---- phase A: projections ---------------------------------
            nb_size = min(512, s)
            for pb in range(4):
                for nb in range(s // nb_size):
                    ps_qk = ps_big.tile([128, nb_size], F32, tag="sc", name="ps_qk")
                    for ec in range(EC):
                        nc.tensor.matmul(
                            ps_qk[:],
                            wqkT_s[:, ec, pb, :],
                            xT_s[:, ec, nb * nb_size : (nb + 1) * nb_size],
                            start=(ec == 0),
                            stop=(ec == EC - 1),
                        )
                    nc.vector.tensor_scalar_add(
                        qkT_s[:, pb, nb * nb_size : (nb + 1) * nb_size],
                        ps_qk[:],
                        bqk_s[:, pb : pb + 1],
                    )

            for st in range(n_st):
                ps_v = ps_big.tile([128, hl * d], F32, tag="sc", name="ps_v")
                for ec in range(EC):
                    nc.tensor.matmul(
                        ps_v[:],
                        xT_s[:, ec, st * 128 : (st + 1) * 128],
                        wvT_s[:, ec, :],
                        start=(ec == 0),
                        stop=(ec == EC - 1),
                    )
                for h in range(hl):
                    nc.vector.tensor_copy(
                        vaug_s[:, st, h, 0:d], ps_v[:, h * d : (h + 1) * d]
                    )

            # ---- phase B: attention -----------------------------------
            for qb in range(n_qb):
                q0 = qb * qb_size
                ps_os = [
                    ps_o.tile([d + 1, qb_size], F32, tag=f"pso{h}", name=f"pso{h}")
                    for h in range(hl)
                ]
                for kc in range(n_kc):
                    nota_t = nota_pool.tile([128, qb_size], mybir.dt.uint16)
                    nc.sync.dma_start(
                        nota_t[:],
                        notaT_d[kc * 128 : (kc + 1) * 128, q0 : q0 + qb_size],
                    )
                    # pad the head stride so multi-head views stay 3D/strided
                    u_t = u_pool.tile([128, hl, qb_size + 8], BF16)
                    for p in range(2):
                        ps_sp = ps_big.tile(
                            [128, 2, qb_size], F32, tag="sc", name=f"ps_s{p}"
                        )
                        for hh in range(2):
                            h = 2 * p + hh
                            p0 = 64 * (h % 2)
                            nc.tensor.matmul(
                                ps_sp[:, hh, :],
                                qkT_s[p0 : p0 + d, 2 + h // 2, kc * 128 : (kc + 1) * 128],
                                qkT_s[p0 : p0 + d, h // 2, q0 : q0 + qb_size],
                                start=True,
                                stop=True,
                            )
                        nc.scalar.activation(
                            u_t[:, 2 * p : 2 * p + 2, :qb_size],
                            ps_sp[:],
                            mybir.ActivationFunctionType.Exp,
                        )
                    nc.vector.copy_predicated(
                        u_t[:, :, :qb_size],
                        nota_t[:, None, :].to_broadcast((128, hl, qb_size)),
                        ones_s[:, None, :qb_size].to_broadcast((128, hl, qb_size)),
                    )
                    for h in range(hl):
                        nc.tensor.matmul(
                            ps_os[h][:],
                            vaug_s[:, kc, h, :],
                            u_t[:, h, :qb_size],
                            start=(kc == 0),
                            stop=(kc == n_kc - 1),
                        )
                # evacuate attn PSUM to SBUF right away so the next q-block's
                # matmuls can claim the banks (keeps the PE stream dense and
                # HAM warm); everything downstream works off the staging copy
                stages = []
                for h in range(hl):
                    stg = small.tile([d + 1, qb_size], F32, tag=f"stg{h}", name=f"stg{h}")
                    nc.vector.tensor_copy(stg[:], ps_os[h][:])
                    stages.append(stg)
                # normalize: out^T[d, q] = num^T / D  (D = staging row d)
                for h in range(hl):
                    stg = stages[h]
                    rec = small.tile([d + 1, qb_size], F32, tag="rec")
                    nc.vector.reciprocal(rec[d : d + 1, :], stg[d : d + 1, :])
                    # replicate 1/D across d partitions via a DRAM bounce
                    # (SBUF APs cannot have partition step 0; DRAM APs can)
                    rd = dbounce.tile([qb_size], F32, tag="rd")
                    nc.sync.dma_start(rd[None, :], rec[d : d + 1, :])
                    repl = small.tile([d, qb_size], F32, tag="repl")
                    nc.sync.dma_start(repl[:], rd[None, :].to_broadcast((d, qb_size)))
                    nc.vector.tensor_tensor(
                        outT_s[:, h, q0 : q0 + qb_size],
                        stg[0:d, :],
                        repl[:],
                        mybir.AluOpType.mult,
                    )
                # projection for the finished q-block
                for j in range(qb_size // 128):
                    st = (q0 // 128) + j
                    ps_p = ps_o.tile([128, e], F32, tag=f"pso{j}", name=f"ps_p{j}")
                    for h in range(hl):
                        nc.tensor.matmul(
                            ps_p[:],
                            outT_s[:, h, st * 128 : (st + 1) * 128],
                            woT_s[:, h, :],
                            start=(h == 0),
                            stop=(h == hl - 1),
                        )
                    oo = outbuf.tile([128, e], F32)
                    nc.vector.tensor_copy(oo[:], ps_p[:])
                    nc.sync.dma_start(part_d[st * 128 : (st + 1) * 128, :], oo[:])

    nc.compile()
    return nc


def _prep_core_inputs(inputs, core):
    """Slice/transpose/cast the full problem inputs for one core."""
    b_i, half = core // 2, core % 2
    g0 = HL * half  # first global head

    x = inputs["x"][b_i]                       # [s, e] f32
    adj = inputs["adj"][b_i]                   # [s, s] f32
    Wqkv_w, Wqkv_b = inputs["Wqkv_w"], inputs["Wqkv_b"]
    out_w = inputs["out_w"]

    scale = 1.0 / np.sqrt(D)

    def head_rows(base, g):
        return slice(base + g * D, base + (g + 1) * D)

    # wqkT pair-blocks + bias
    blocks, brows = [], []
    for pb in range(4):
        if pb < 2:  # q blocks, pre-scaled
            g_a, g_b = g0 + 2 * pb, g0 + 2 * pb + 1
            wa = Wqkv_w[head_rows(0, g_a)] * scale
            wb = Wqkv_w[head_rows(0, g_b)] * scale
            ba = Wqkv_b[head_rows(0, g_a)] * scale
            bb = Wqkv_b[head_rows(0, g_b)] * scale
        else:       # k blocks
            g_a, g_b = g0 + 2 * (pb - 2), g0 + 2 * (pb - 2) + 1
            wa = Wqkv_w[head_rows(E, g_a)]
            wb = Wqkv_w[head_rows(E, g_b)]
            ba = Wqkv_b[head_rows(E, g_a)]
            bb = Wqkv_b[head_rows(E, g_b)]
        blocks.append(np.concatenate([wa, wb], axis=0).T)   # [e, 128]
        brows.append(np.concatenate([ba, bb], axis=0))      # [128]
    wqkT = np.stack(blocks, axis=1)                          # [e, 4, 128]
    bqk = np.stack(brows, axis=1)                            # [128, 4]

    # v weights, local-head-major columns: [e, hl*d]
    wv_rows = np.concatenate(
        [Wqkv_w[head_rows(2 * E, g0 + h)] for h in range(HL)], axis=0
    )                                                        # [hl*d, e]
    wvT = wv_rows.T                                          # [e, hl*d]

    # out projection slice, per local head: [d, hl, e]
    woT = np.stack(
        [out_w[:, (g0 + h) * D : (g0 + h + 1) * D].T for h in range(HL)], axis=1
    )

    notaT = np.ascontiguousarray((1.0 - adj).T)

    bf = np.float32  # cast via ml_dtypes-compatible numpy path below
    import ml_dtypes

    def c(a):
        return np.ascontiguousarray(a.astype(ml_dtypes.bfloat16))

    return {
        "xT": c(x.T),
        "wqkT": c(wqkT),
        "bqk": np.ascontiguousarray(bqk.astype(np.float32)),
        "wvT": c(wvT),
        "woT": c(woT),
        "notaT": np.ascontiguousarray(notaT.astype(np.uint16)),
    }


def run(inputs, **spmd_kwargs):
    """Run the 8-core kernel; returns (full output, BassKernelResults)."""
    global _CACHED_NC
    if _CACHED_NC is None:
        _CACHED_NC = build_kernel()
    nc = _CACHED_NC

    in_maps = [_prep_core_inputs(inputs, c) for c in range(N_CORES)]
    res = run_bass_kernel_spmd(
        nc, in_maps, core_ids=list(range(N_CORES)), **spmd_kwargs
    )

    # host-side combine: sum head-half partials, add folded bias
    out_w = inputs["out_w"].astype(np.float64)
    out_b = inputs["out_b"].astype(np.float64)
    bv = inputs["Wqkv_b"][2 * E : 3 * E].astype(np.float64)
    bias_full = (out_b + bv @ out_w.T).astype(np.float32)    # [e]

    out = np.empty((B, S, E), dtype=np.float32)
    for b_i in range(B):
        p0 = res.results[2 * b_i]["part"]
        p1 = res.results[2 * b_i + 1]["part"]
        out[b_i] = p0 + p1 + bias_full
    return out, res


def kernel(**inputs):
    return run(inputs)[0]


# revision 18
# speedup vs baseline: 1.1291x; 1.1291x over previous
"""Sparse (adjacency-masked) multi-head attention for Trainium2, 8 cores.

Problem: b=4, s=2048, e=512, h=8 heads, d=64.
  qkv = x @ Wqkv^T + b -> q,k,v per head
  scores = (q @ k^T) / sqrt(d) * adj   (multiplicative 0/1 mask, clip is a no-op)
  attn = softmax(scores); out = (attn @ v) reshaped @ out_w^T + out_b

Sharding: core c -> batch c//2, local heads [4*(c%2), 4*(c%2)+4).  Each core
computes a partial out-projection over its 4 heads; host sums the two
partials per batch and adds the (host-folded) biases.  No collectives.

Key device-side formulation (all matmuls bf16 -> fp32 PSUM):
  - Everything is computed transposed: S^T[k,q] = k^T(stationary) . q^T so the
    mask ships host-transposed; U^T = exp(S^T) with masked entries forced to
    exp(0)=1 via copy_predicated (matches reference: masked score 0 -> exp 1).
  - attn numerator+denominator in one matmul: lhsT = [v_h | 1] (M=65), so PSUM
    row 64 is the softmax denominator.
  - v bias never touches the device: softmax rows sum to 1, so +bv shifts the
    attention output by a constant vector; it is folded into the final bias on
    host as (bv_full @ out_w^T) + out_b.
"""

import numpy as np

import concourse.bass as bass
import concourse.tile as tile
from concourse import bacc, mybir
from concourse.bass_utils import run_bass_kernel_spmd

BF16 = mybir.dt.bfloat16
F32 = mybir.dt.float32

# Problem constants (hardcoded per contract)
B, S, E = 4, 2048, 512
H_TOT, D = 8, 64
HL = 4            # local heads per core
N_CORES = 8

_CACHED_NC = None


def _pin_act_table_set():
    """Both Exp and Ln live in the 'natural_log_exp_and_others' ACT table set.
    By default walrus homes Exp in 'exp_and_others', so a kernel using Exp+Ln
    reloads tables (~2.7us) every switch. Point the compiler at a filtered
    act_info.json exposing only the combined set so one load serves both."""
    import json
    import os
    import tempfile

    if os.environ.get("BASS_ACT_ROOT_JSON_PATH"):
        return
    try:
        from neuronxcc.driver.Job import Job
        from neuronxcc.driver.jobs.support.FindActInfo import findActInfoFile

        src = findActInfoFile(Job.getPackageDir(), "gen3")
        srcdir = os.path.dirname(src)
        d = json.load(open(src))
        d["act_func_sets"] = [
            s for s in d["act_func_sets"]
            if s["name"] == "natural_log_exp_and_others"
        ]
        assert d["act_func_sets"]
        tmpdir = tempfile.mkdtemp(prefix="act_pin_")
        for fn in os.listdir(srcdir):
            if fn != "act_info.json":
                os.symlink(os.path.join(srcdir, fn), os.path.join(tmpdir, fn))
        out = os.path.join(tmpdir, "act_info.json")
        with open(out, "w") as f:
            json.dump(d, f)
        os.environ["BASS_ACT_ROOT_JSON_PATH"] = out

        # bass's insert_act_table_loads indexes the same json walrus reads;
        # point its table getter at the filtered file so the set ids match
        import concourse.bacc as _bacc
        import concourse.mybir as _mybir

        def _tables(_arch):
            return {
                ent["name"]: {
                    _mybir.ActivationFunctionType.from_pwp(v)
                    for v in ent["act"].keys()
                }
                for ent in d["act_func_sets"]
            }

        _bacc.get_activation_tables = _tables
    except Exception:
        pass  # fall back to stock tables (correct, just slower)


def build_kernel(s=S, e=E, hl=HL, d=D, qb_size=512):
    """Per-core SPMD kernel. Inputs (per core):
      xT     [e, s]        bf16  (x[b].T)
      wqkT   [e, 4, 128]   bf16  pair-blocks pb: 0=[q_h0;q_h1] 1=[q_h2;q_h3]
                                 2=[k_h0;k_h1] 3=[k_h2;k_h3]; q pre-scaled 1/sqrt(d)
      bqk    [128, 4]      f32   bias rows matching wqkT blocks (q pre-scaled)
      wvT    [e, hl*d]     bf16  v weights, local-head-major columns
      woT    [d, hl, e]    bf16  out_w slice transposed per local head
      notaT  [s, s]        bf16  (1 - adj[b]).T  (indexed [k, q])
    Output:
      part   [s, e]        f32   partial out-projection (no bias)
    """
    assert e % 128 == 0 and s % 128 == 0
    EC = e // 128                 # contraction chunks for projections
    n_qb = s // qb_size           # q blocks
    n_kc = s // 128               # k chunks
    n_st = s // 128               # s tiles for v / proj

    _pin_act_table_set()
    nc = bacc.Bacc(None, target_bir_lowering=False)

    xT_d = nc.dram_tensor("xT", [e, s], BF16, kind="ExternalInput")
    wqkT_d = nc.dram_tensor("wqkT", [e, 4, 128], BF16, kind="ExternalInput")
    bqk_d = nc.dram_tensor("bqk", [128, 4], F32, kind="ExternalInput")
    wvT_d = nc.dram_tensor("wvT", [e, hl * d], BF16, kind="ExternalInput")
    woT_d = nc.dram_tensor("woT", [d, hl, e], BF16, kind="ExternalInput")
    notaT_d = nc.dram_tensor("notaT", [s, s], mybir.dt.uint16, kind="ExternalInput")
    part_d = nc.dram_tensor("part", [s, e], F32, kind="ExternalOutput")

    with tile.TileContext(nc) as tc:
        with (
            tc.tile_pool(name="singles", bufs=1) as singles,
            tc.tile_pool(name="nota", bufs=3) as nota_pool,
            tc.tile_pool(name="upool", bufs=3) as u_pool,
            tc.tile_pool(name="small", bufs=4) as small,
            tc.tile_pool(name="dbounce", bufs=4, space="DRAM") as dbounce,
            tc.tile_pool(name="outbuf", bufs=3) as outbuf,
            tc.tile_pool(name="ps_big", bufs=1, space="PSUM") as ps_big,
            tc.tile_pool(name="ps_o", bufs=1, space="PSUM") as ps_o,
        ):
            # ---- resident tensors -------------------------------------
            xT_s = singles.tile([128, EC, s], BF16)
            nc.sync.dma_start(xT_s[:], xT_d.rearrange("(eo ei) s -> ei eo s", ei=128))
            wqkT_s = singles.tile([128, EC, 4, 128], BF16)
            nc.sync.dma_start(
                wqkT_s[:], wqkT_d.rearrange("(eo ei) pb j -> ei eo pb j", ei=128)
            )
            bqk_s = singles.tile([128, 4], F32)
            nc.sync.dma_start(bqk_s[:], bqk_d[:])
            wvT_s = singles.tile([128, EC, hl * d], BF16)
            nc.sync.dma_start(
                wvT_s[:], wvT_d.rearrange("(eo ei) f -> ei eo f", ei=128)
            )
            woT_s = singles.tile([d, hl, e], BF16)
            nc.sync.dma_start(woT_s[:], woT_d[:])

            ones_s = singles.tile([128, 4 * qb_size], BF16)
            nc.vector.memset(ones_s[:], 1.0)

            # qkT pair-blocks: [128, 4, s]; head h lives at partitions
            # 64*(h%2) .. +64 of block (h//2) [q] / 2+(h//2) [k]
            qkT_s = singles.tile([128, 4, s], BF16)
            # v augmented with a ones column: [128, st, h, d+1]
            vaug_s = singles.tile([128, n_st, hl, d + 1], BF16)
            nc.vector.memset(vaug_s[:], 1.0)
            # normalized attn output, transposed: [d, h, s] (partitions 0:d)
            outT_s = singles.tile([d, hl, s], BF16)

            # ---- phase A: projections ---------------------------------
            nb_size = min(512, s)
            for pb in range(4):
                for nb in range(s // nb_size):
                    ps_qk = ps_big.tile([128, nb_size], F32, tag="sc", name="ps_qk")
                    for ec in range(EC):
                        nc.tensor.matmul(
                            ps_qk[:],
                            wqkT_s[:, ec, pb, :],
                            xT_s[:, ec, nb * nb_size : (nb + 1) * nb_size],
                            start=(ec == 0),
                            stop=(ec == EC - 1),
                        )
                    nc.vector.tensor_scalar_add(
                        qkT_s[:, pb, nb * nb_size : (nb + 1) * nb_size],
                        ps_qk[:],
                        bqk_s[:, pb : pb + 1],
                    )

            for st in range(n_st):
                ps_v = ps_big.tile([128, hl * d], F32, tag="sc", name="ps_v")
                for ec in range(EC):
                    nc.tensor.matmul(
                        ps_v[:],
                        xT_s[:, ec, st * 128 : (st + 1) * 128],
                        wvT_s[:, ec, :],
                        start=(ec == 0),
                        stop=(ec == EC - 1),
                    )
                for h in range(hl):
                    nc.vector.tensor_copy(
                        vaug_s[:, st, h, 0:d], ps_v[:, h * d : (h + 1) * d]
                    )

            # ---- phase B: attention -----------------------------------
            for qb in range(n_qb):
                q0 = qb * qb_size
                ps_os = [
                    ps_o.tile([d + 1, qb_size], F32, tag=f"pso{h}", name=f"pso{h}")
                    for h in range(hl)
                ]
                for kc in range(n_kc):
                    nota_t = nota_pool.tile([128, qb_size], mybir.dt.uint16)
                    nc.sync.dma_start(
                        nota_t[:],
                        notaT_d[kc * 128 : (kc + 1) * 128, q0 : q0 + qb_size],
                    )
                    # pad the head stride so multi-head views stay 3D/strided
                    u_t = u_pool.tile([128, hl, qb_size + 8], BF16)
                    for p in range(2):
                        ps_sp = ps_big.tile(
                            [128, 2, qb_size], F32, tag="sc", name=f"ps_s{p}"
                        )
                        for hh in range(2):
                            h = 2 * p + hh
                            p0 = 64 * (h % 2)
                            nc.tensor.matmul(
                                ps_sp[:, hh, :],
                                qkT_s[p0 : p0 + d, 2 + h // 2, kc * 128 : (kc + 1) * 128],
                                qkT_s[p0 : p0 + d, h // 2, q0 : q0 + qb_size],
                                start=True,
                                stop=True,
                            )
                        nc.scalar.activation(
                            u_t[:, 2 * p : 2 * p + 2, :qb_size],
                            ps_sp[:],
                            mybir.ActivationFunctionType.Exp,
                        )
                    nc.vector.copy_predicated(
                        u_t[:, :, :qb_size],
                        nota_t[:, None, :].to_broadcast((128, hl, qb_size)),
                        ones_s[:, None, :qb_size].to_broadcast((128, hl, qb_size)),
                    )
                    for h in range(hl):
                        nc.tensor.matmul(
                            ps_os[h][:],
                            vaug_s[:, kc, h, :],
                            u_t[:, h, :qb_size],
                            start=(kc == 0),
                            stop=(kc == n_kc - 1),
                        )
                # evacuate attn PSUM to SBUF right away so the next q-block's
                # matmuls can claim the banks (keeps the PE stream dense and
                # HAM warm); everything downstream works off the staging copy
                stages = []
                for h in range(hl):
                    stg = small.tile([d + 1, qb_size], F32, tag=f"stg{h}", name=f"stg{h}")
                    nc.vector.tensor_copy(stg[:], ps_os[h][:])
                    stages.append(stg)
                # normalize: out^T[d, q] = num^T / D  (D = staging row d)
                for h in range(hl):
                    stg = stages[h]
                    # 1/D = exp(-ln(D)) on ScalarE: keeps the in-order DVE
                    # queue free for the next q-block's copy_predicated
                    rec = small.tile([d + 1, 2, qb_size], F32, tag="rec")
                    nc.scalar.activation(
                        rec[d : d + 1, 0, :],
                        stg[d : d + 1, :],
                        mybir.ActivationFunctionType.Ln,
                    )
                    nc.scalar.activation(
                        rec[d : d + 1, 1, :],
                        rec[d : d + 1, 0, :],
                        mybir.ActivationFunctionType.Exp,
                        scale=-1.0,
                    )
                    # replicate 1/D across d partitions via a DRAM bounce
                    # (SBUF APs cannot have partition step 0; DRAM APs can)
                    rd = dbounce.tile([qb_size], F32, tag="rd")
                    nc.sync.dma_start(rd[None, :], rec[d : d + 1, 1, :])
                    repl = small.tile([d, qb_size], F32, tag="repl")
                    nc.sync.dma_start(repl[:], rd[None, :].to_broadcast((d, qb_size)))
                    nc.vector.tensor_tensor(
                        outT_s[:, h, q0 : q0 + qb_size],
                        stg[0:d, :],
                        repl[:],
                        mybir.AluOpType.mult,
                    )
                # projection for the finished q-block
                for j in range(qb_size // 128):
                    st = (q0 // 128) + j
                    ps_p = ps_o.tile([128, e], F32, tag=f"pso{j}", name=f"ps_p{j}")
                    for h in range(hl):
                        nc.tensor.matmul(
                            ps_p[:],
                            outT_s[:, h, st * 128 : (st + 1) * 128],
                            woT_s[:, h, :],
                            start=(h == 0),
                            stop=(h == hl - 1),
                        )
                    oo = outbuf.tile([128, e], F32)
                    nc.vector.tensor_copy(oo[:], ps_p[:])
                    nc.sync.dma_start(part_d[st * 128 : (st + 1) * 128, :], oo[:])

    nc.compile()
    return nc


def _prep_core_inputs(inputs, core):
    """Slice/transpose/cast the full problem inputs for one core."""
    b_i, half = core // 2, core % 2
    g0 = HL * half  # first global head

    x = inputs["x"][b_i]                       # [s, e] f32
    adj = inputs["adj"][b_i]                   # [s, s] f32
    Wqkv_w, Wqkv_b = inputs["Wqkv_w"], inputs["Wqkv_b"]
    out_w = inputs["out_w"]

    scale = 1.0 / np.sqrt(D)

    def head_rows(base, g):
        return slice(base + g * D, base + (g + 1) * D)

    # wqkT pair-blocks + bias
    blocks, brows = [], []
    for pb in range(4):
        if pb < 2:  # q blocks, pre-scaled
            g_a, g_b = g0 + 2 * pb, g0 + 2 * pb + 1
            wa = Wqkv_w[head_rows(0, g_a)] * scale
            wb = Wqkv_w[head_rows(0, g_b)] * scale
            ba = Wqkv_b[head_rows(0, g_a)] * scale
            bb = Wqkv_b[head_rows(0, g_b)] * scale
        else:       # k blocks
            g_a, g_b = g0 + 2 * (pb - 2), g0 + 2 * (pb - 2) + 1
            wa = Wqkv_w[head_rows(E, g_a)]
            wb = Wqkv_w[head_rows(E, g_b)]
            ba = Wqkv_b[head_rows(E, g_a)]
            bb = Wqkv_b[head_rows(E, g_b)]
        blocks.append(np.concatenate([wa, wb], axis=0).T)   # [e, 128]
        brows.append(np.concatenate([ba, bb], axis=0))      # [128]
    wqkT = np.stack(blocks, axis=1)                          # [e, 4, 128]
    bqk = np.stack(brows, axis=1)                            # [128, 4]

    # v weights, local-head-major columns: [e, hl*d]
    wv_rows = np.concatenate(
        [Wqkv_w[head_rows(2 * E, g0 + h)] for h in range(HL)], axis=0
    )                                                        # [hl*d, e]
    wvT = wv_rows.T                                          # [e, hl*d]

    # out projection slice, per local head: [d, hl, e]
    woT = np.stack(
        [out_w[:, (g0 + h) * D : (g0 + h + 1) * D].T for h in range(HL)], axis=1
    )

    notaT = np.ascontiguousarray((1.0 - adj).T)

    bf = np.float32  # cast via ml_dtypes-compatible numpy path below
    import ml_dtypes

    def c(a):
        return np.ascontiguousarray(a.astype(ml_dtypes.bfloat16))

    return {
        "xT": c(x.T),
        "wqkT": c(wqkT),
        "bqk": np.ascontiguousarray(bqk.astype(np.float32)),
        "wvT": c(wvT),
        "woT": c(woT),
        "notaT": np.ascontiguousarray(notaT.astype(np.uint16)),
    }


def run(inputs, **spmd_kwargs):
    """Run the 8-core kernel; returns (full output, BassKernelResults)."""
    global _CACHED_NC
    if _CACHED_NC is None:
        _CACHED_NC = build_kernel()
    nc = _CACHED_NC

    in_maps = [_prep_core_inputs(inputs, c) for c in range(N_CORES)]
    res = run_bass_kernel_spmd(
        nc, in_maps, core_ids=list(range(N_CORES)), **spmd_kwargs
    )

    # host-side combine: sum head-half partials, add folded bias
    out_w = inputs["out_w"].astype(np.float64)
    out_b = inputs["out_b"].astype(np.float64)
    bv = inputs["Wqkv_b"][2 * E : 3 * E].astype(np.float64)
    bias_full = (out_b + bv @ out_w.T).astype(np.float32)    # [e]

    out = np.empty((B, S, E), dtype=np.float32)
    for b_i in range(B):
        p0 = res.results[2 * b_i]["part"]
        p1 = res.results[2 * b_i + 1]["part"]
        out[b_i] = p0 + p1 + bias_full
    return out, res


def kernel(**inputs):
    return run(inputs)[0]
